# revision 21
# baseline (speedup 1.0000x reference)
"""DiscGCN (3-layer GCN, 100k nodes / 1.6M edges) on 8 Trainium2 NeuronCores.

Strategy: nodes + incident (dst) edges sharded across 8 cores. Per layer the
transformed features (pre-scaled by dinv[src]) are AllGathered in
int16-addressable chunks into a replicated DRAM table; per-edge messages are
fetched with dma_gather round-robined over 4 SWDGE queues and aggregated with
one-hot matmuls. Edges are ordered band-major (bands of 16 dst strips,
chunk-major inside a band) so gather segments stay coarse while each dst
strip accumulates in a single PSUM tile across all its groups (self loop
included via an identity-A matmul on rows DMAd from the bounce buffer).
Per-strip epilogue applies the dst-side dinv and ReLU+bias, letting the next
layer's produce/AllGather overlap the current message passing.
Tables/messages bf16; accumulation f32.
"""
import numpy as np

import concourse.bacc as bacc
import concourse.bass as bass
import concourse.tile as tile
from concourse import mybir
from concourse.bass_utils import run_bass_kernel_spmd

N_CORES = 8
D = 128
P = 128

N_NODES = 100000
NLOC = 12500

N_QUEUES = 4
BAND = 6  # strips per band (= live PSUM accumulator banks)


def build(ei, n_nodes, nloc, batch_groups=64):
    nlocp = ((nloc + P - 1) // P) * P
    n_strips = nlocp // P
    vtab = N_CORES * nlocp
    n_chunks = max(1, int(np.ceil(vtab / 32768 + 1e-9)))
    sl = int(np.ceil(nlocp / n_chunks / P)) * P
    while N_CORES * sl > 32767:
        n_chunks += 1
        sl = int(np.ceil(nlocp / n_chunks / P)) * P
    chunk_off = [min(j * sl, nlocp) for j in range(n_chunks + 1)]
    chunk_len = [chunk_off[j + 1] - chunk_off[j] for j in range(n_chunks)]
    coff = np.array(chunk_off[:-1])

    src = np.asarray(ei[0], dtype=np.int64)
    dst = np.asarray(ei[1], dtype=np.int64)

    deg = 1.0 + np.bincount(dst, minlength=n_nodes).astype(np.float32)
    dinv = (1.0 / np.sqrt(deg)).astype(np.float32)

    src_core = src // nloc
    src_loc = src % nloc
    src_chunk = np.minimum(src_loc // sl, n_chunks - 1)
    src_idx = (src_core * np.array(chunk_len)[src_chunk]
               + (src_loc - coff[src_chunk]))
    src_rowabs = N_CORES * coff[src_chunk] + src_idx

    core = dst // nloc
    dloc = dst % nloc
    strip = dloc // P
    dstp = dloc % P

    n_bands = (n_strips + BAND - 1) // BAND

    per_core = []
    for c in range(N_CORES):
        m = core == c
        e_idx = src_idx[m]
        e_rowabs = src_rowabs[m]
        e_s = strip[m]
        e_dstp = dstp[m]
        e_chunk = src_chunk[m]
        e_band = e_s // BAND
        order = np.lexsort((e_idx, e_s, e_chunk, e_band))
        per_core.append((e_chunk[order], e_s[order], e_idx[order],
                         e_rowabs[order], e_dstp[order]))

    run_lens = np.zeros((N_CORES, n_chunks, n_strips), np.int64)
    for c in range(N_CORES):
        e_chunk, e_s = per_core[c][0], per_core[c][1]
        key = e_chunk * n_strips + e_s
        cnt = np.bincount(key, minlength=n_chunks * n_strips).reshape(
            n_chunks, n_strips)
        run_lens[c] = cnt
    run_groups = (run_lens.max(axis=0) + P - 1) // P  # [n_chunks, n_strips]

    # stream order: band-major, chunk-major inside band, strip inside chunk
    grid = []  # (chunk, strip, group_start, n_groups)
    gstart = 0
    for bd in range(n_bands):
        s0, s1 = bd * BAND, min((bd + 1) * BAND, n_strips)
        for ck in range(n_chunks):
            for s in range(s0, s1):
                ng = int(run_groups[ck, s])
                if ng == 0:
                    continue
                grid.append((ck, s, gstart, ng))
                gstart += ng
    n_groups_reg = gstart
    n_groups_pad = ((n_groups_reg + batch_groups - 1)
                    // batch_groups) * batch_groups
    n_batches = n_groups_pad // batch_groups
    nslot = n_groups_pad * P

    idx16 = np.full((N_CORES, nslot), -1, np.int16)
    slot_dstp = np.full((N_CORES, nslot), -1.0, np.float32)
    slot_row_abs = np.zeros((N_CORES, nslot), np.int64)
    slot_strip = np.full(nslot, -1, np.int64)

    group_chunk = np.zeros(n_groups_pad, np.int64)
    for (ck, s, g0, ng) in grid:
        group_chunk[g0:g0 + ng] = ck
        slot_strip[g0 * P:(g0 + ng) * P] = s
    if grid and n_groups_reg < n_groups_pad:
        group_chunk[n_groups_reg:n_groups_pad] = grid[-1][0]

    base_of_chunk = N_CORES * coff
    for c in range(N_CORES):
        e_chunk, e_s, e_idx, e_rowabs, e_dstp = per_core[c]
        pos = 0
        for (ck, s, g0, ng) in grid:
            ln = int(run_lens[c, ck, s])
            sl0 = g0 * P
            idx16[c, sl0:sl0 + ln] = e_idx[pos:pos + ln].astype(np.int16)
            slot_row_abs[c, sl0:sl0 + ln] = e_rowabs[pos:pos + ln]
            slot_dstp[c, sl0:sl0 + ln] = e_dstp[pos:pos + ln].astype(np.float32)
            pos += ln
        assert pos == len(e_idx)
        # pad slots gather row 0 of their chunk (valid, finite); dstp stays -1
        mpad = idx16[c] < 0
        gch = group_chunk[np.arange(nslot) // P]
        idx16[c][mpad] = 0
        slot_row_abs[c][mpad] = base_of_chunk[gch][mpad]

    # gather segments: per batch, consecutive groups sharing a chunk, <=32
    segments = []  # (batch, glo, ghi, chunk, seg_id)
    for b in range(n_batches):
        glo = b * batch_groups
        ghi_b = min(n_groups_pad, (b + 1) * batch_groups)
        g = glo
        while g < ghi_b:
            ck = group_chunk[g]
            h = g
            while h < ghi_b and group_chunk[h] == ck and h - g < 32:
                h += 1
            sid = len(segments)
            segments.append((b, g - b * batch_groups, h - b * batch_groups,
                             int(ck), sid))
            g = h

    # device oplist: per band emit selfs, then that band's runs (split at
    # batch boundaries), with per-strip stop flags (epilogue points).
    oplist = []
    last_piece = {}
    run_pieces_of_band = [[] for _ in range(n_bands)]
    for (ck, s, g0, ng) in grid:
        g = g0
        while g < g0 + ng:
            b = g // batch_groups
            ghi = min(g0 + ng, (b + 1) * batch_groups)
            run_pieces_of_band[s // BAND].append(
                ('run', s, b, g - b * batch_groups, ghi - b * batch_groups))
            g = ghi
    for bd in range(n_bands):
        s0, s1 = bd * BAND, min((bd + 1) * BAND, n_strips)
        for s in range(s0, s1):
            oplist.append(['self', s, 0, 0, 0, False])
            last_piece[s] = len(oplist) - 1
        for pc in run_pieces_of_band[bd]:
            oplist.append(list(pc) + [False])
            last_piece[pc[1]] = len(oplist) - 1
    for s, i in last_piece.items():
        oplist[i][5] = True  # stop flag -> epilogue follows this piece
    oplist = [tuple(x) for x in oplist]

    bg16 = batch_groups * P // 16
    idx_wrapped = np.zeros((N_CORES, 128, n_batches * bg16), np.int16)
    for c in range(N_CORES):
        for b in range(n_batches):
            fl = idx16[c, b * batch_groups * P:(b + 1) * batch_groups * P]
            w = fl.reshape(-1, 16).T
            idx_wrapped[c, :, b * bg16:(b + 1) * bg16] = np.tile(w, (8, 1))

    dstp_g = np.zeros((N_CORES, 128, n_groups_pad), np.float32)
    for c in range(N_CORES):
        dstp_g[c] = slot_dstp[c].reshape(n_groups_pad, P).T

    # per-core dinv in slab layout [P, n_strips] and replicated row layout
    dinvT = np.zeros((N_CORES, P, n_strips), np.float32)
    dinv_row = np.zeros((N_CORES, 1, nlocp), np.float32)
    for c in range(N_CORES):
        dl = np.zeros(nlocp, np.float32)
        r = dinv[c * nloc:(c + 1) * nloc]
        dl[:r.shape[0]] = r
        dinvT[c] = dl.reshape(n_strips, P).T
        dinv_row[c, 0] = dl

    # self identity-A patterns: [P, 2, P] bf16; pattern 0 full identity,
    # pattern 1 partial (strip n_strips-1, valid rows only)
    selfA = np.zeros((P, 2, P), np.float32)
    selfA[:, 0, :] = np.eye(P, dtype=np.float32)
    nvalid_last = nloc - (n_strips - 1) * P
    pe = np.eye(P, dtype=np.float32)
    pe[max(0, nvalid_last):] = 0.0
    selfA[:, 1, :] = pe

    return dict(
        nlocp=nlocp, n_strips=n_strips, n_bands=n_bands,
        vtab=vtab, n_chunks=n_chunks, chunk_off=chunk_off, chunk_len=chunk_len,
        n_batches=n_batches, batch_groups=batch_groups, nslot=nslot,
        n_groups_pad=n_groups_pad, grid=grid, group_chunk=group_chunk,
        segments=segments, oplist=oplist, slot_strip=slot_strip,
        idx_wrapped=idx_wrapped, dstp_g=dstp_g,
        slot_row_abs=slot_row_abs, slot_dstp=slot_dstp,
        dinv=dinv, dinvT=dinvT, dinv_row=dinv_row, selfA=selfA,
    )


def host_simulate(ei, x, Ws, bs, meta, nloc):
    """Vectorized numpy simulation of the device algorithm (layout check)."""
    nlocp, vtab = meta["nlocp"], meta["vtab"]
    slot_strip = meta["slot_strip"]
    dinv = meta["dinv"]
    chunk_off, chunk_len = meta["chunk_off"], meta["chunk_len"]

    def to_table(h_list):
        """chunk-interleaved layout: chunk j holds rows
        [8*off_j + c*len_j + (loc-off_j)] for core c, local row loc."""
        t = np.zeros((vtab, h_list[0].shape[1]), np.float32)
        for j in range(len(chunk_len)):
            o, ln = chunk_off[j], chunk_len[j]
            for c in range(N_CORES):
                base = N_CORES * o + c * ln
                t[base:base + ln] = h_list[c][o:o + ln]
        return t

    x_loc = []
    dinv_loc = []
    for c in range(N_CORES):
        xx = np.zeros((nlocp, x.shape[1]), np.float32)
        r = x[c * nloc:(c + 1) * nloc]
        xx[:r.shape[0]] = r
        x_loc.append(xx)
        dl = np.zeros((nlocp,), np.float32)
        dl[:r.shape[0]] = dinv[c * nloc:c * nloc + r.shape[0]]
        dinv_loc.append(dl)

    h = x_loc
    outs = []
    for l, (W, b) in enumerate(zip(Ws, bs)):
        last = l == len(Ws) - 1
        if not last:
            t = to_table([dinv_loc[c][:, None] * (h[c] @ W)
                          for c in range(N_CORES)])
            rows_loc = [dinv_loc[c][:, None] * (h[c] @ W) for c in range(N_CORES)]
        else:
            t = to_table([dinv_loc[c][:, None] * h[c] for c in range(N_CORES)])
            rows_loc = [dinv_loc[c][:, None] * h[c] for c in range(N_CORES)]
        hn = []
        for c in range(N_CORES):
            rows = meta["slot_row_abs"][c]
            msgs = t[rows]
            dd = meta["slot_dstp"][c]
            valid = (dd >= 0) & (slot_strip >= 0)
            a = np.zeros((nlocp, t.shape[1]), np.float32)
            tgt = slot_strip * P + dd.astype(np.int64)
            np.add.at(a, tgt[valid], msgs[valid])
            a = a + rows_loc[c]  # self contribution (dinv-scaled rows)
            a[nloc:] = 0.0
            a = a * dinv_loc[c][:, None]
            if not last:
                hn.append(np.maximum(a + b[None, :], 0.0))
            else:
                outs.append(1.0 / (1.0 + np.exp(-(a @ W + b)))[:nloc])
        if not last:
            h = hn
    return np.concatenate(outs, axis=0)


f32 = mybir.dt.float32
bf16 = mybir.dt.bfloat16
f16 = mybir.dt.float16
i16 = mybir.dt.int16
AF = mybir.ActivationFunctionType
ALU = mybir.AluOpType


def build_program(meta, use_cc=True, mdt=bf16):
    """meta: from build(). Returns finalized nc."""
    nlocp = meta["nlocp"]
    n_strips = meta["n_strips"]
    vtab = meta["vtab"]
    chunk_off = meta["chunk_off"]
    chunk_len = meta["chunk_len"]
    nb = meta["n_batches"]
    bg = meta["batch_groups"]

    segments = meta["segments"]          # (batch, glo, ghi, chunk, seg_id)
    oplist = meta["oplist"]              # (kind, strip, b, glo, ghi, stop)
    bg16 = bg * P // 16

    nc = bacc.Bacc(None, target_bir_lowering=False, num_swdge_queues=N_QUEUES)

    # ---- I/O ----
    xT = nc.dram_tensor("xT", [P, nlocp], f32, kind="ExternalInput")
    W1 = nc.dram_tensor("W1", [P, D], f32, kind="ExternalInput")
    W2 = nc.dram_tensor("W2", [P, D], f32, kind="ExternalInput")
    W3 = nc.dram_tensor("W3", [P, 1], f32, kind="ExternalInput")
    b1 = nc.dram_tensor("b1", [P, 1], f32, kind="ExternalInput")
    b2 = nc.dram_tensor("b2", [P, 1], f32, kind="ExternalInput")
    b3 = nc.dram_tensor("b3", [P, 1], f32, kind="ExternalInput")
    iota = nc.dram_tensor("iota", [P, P], f16, kind="ExternalInput")
    idxd = nc.dram_tensor("idxd", [P, nb * bg16], i16, kind="ExternalInput")
    dstpd = nc.dram_tensor("dstpd", [P, nb * bg], f16, kind="ExternalInput")
    dinvTd = nc.dram_tensor("dinvTd", [P, n_strips], f32, kind="ExternalInput")
    dinvRd = nc.dram_tensor("dinvRd", [P, nlocp], f32, kind="ExternalInput")
    selfAd = nc.dram_tensor("selfAd", [P, 2 * P], f32, kind="ExternalInput")
    out = nc.dram_tensor("out", [nlocp, 1], f32, kind="ExternalOutput")

    # ---- internal DRAM ----
    n_sl = len(chunk_len)
    bounce = [[nc.dram_tensor(f"bounce{l}_{j}", [chunk_len[j], D], mdt)
               for j in range(n_sl)] for l in range(3)]

    _co = {}

    def chunk_off_of(bt):
        return _co[bt.name]

    def bounce_rows(l, r0, r1):
        """list of (tensor, lo, hi) covering local rows [r0, r1)."""
        parts = []
        for j in range(n_sl):
            o = chunk_off[j]
            lo = max(r0, o)
            hi = min(r1, o + chunk_len[j])
            if lo < hi:
                parts.append((bounce[l][j], lo - o, hi - o))
        return parts
    for l in range(3):
        for j in range(n_sl):
            _co[bounce[l][j].name] = chunk_off[j]
    tbl = [nc.dram_tensor(f"tbl{l}", [vtab, D], mdt,
                          addr_space="Shared" if use_cc else "Local")
           for l in range(3)]

    with tile.TileContext(nc) as tc:
        with (
            tc.tile_pool(name="const", bufs=1) as cpool,
            tc.tile_pool(name="slab", bufs=1) as slab_pool,
            tc.tile_pool(name="stream", bufs=4) as spool,
            tc.tile_pool(name="abuf", bufs=6) as apool,
            tc.tile_pool(name="msg", bufs=6) as mpool,
            tc.tile_pool(name="smtp", bufs=6) as smt_pool,
            tc.tile_pool(name="dbcp", bufs=24) as dbc_pool,
            tc.tile_pool(name="stage", bufs=2) as stpool,
            tc.tile_pool(name="accp", bufs=7, space="PSUM") as acc_pool,
            tc.tile_pool(name="gp", bufs=1, space="PSUM") as gp_pool,
        ):
            # constants
            iota_t = cpool.tile([P, P], f16)
            nc.sync.dma_start(out=iota_t[:], in_=iota[:])
            w_t = []
            for l, W in enumerate((W1, W2)):
                t = cpool.tile([P, D], f32, tag=f"w{l}")
                nc.sync.dma_start(out=t[:], in_=W[:])
                w_t.append(t)
            w3_t = cpool.tile([P, 1], f32)
            nc.sync.dma_start(out=w3_t[:], in_=W3[:])
            b_t = []
            for l, B in enumerate((b1, b2)):
                t = cpool.tile([P, 1], f32, tag=f"b{l}")
                nc.sync.dma_start(out=t[:], in_=B[:])
                b_t.append(t)
            b3_t = cpool.tile([P, 1], f32)
            nc.sync.dma_start(out=b3_t[:], in_=b3[:])
            dinvT_t = cpool.tile([P, n_strips], f32)
            nc.sync.dma_start(out=dinvT_t[:], in_=dinvTd[:])
            selfA_f32 = cpool.tile([P, 2, P], f32)
            nc.sync.dma_start(
                out=selfA_f32[:],
                in_=selfAd[:].rearrange("p (two q) -> p two q", two=2))
            selfA_t = cpool.tile([P, 2, P], mdt)
            nc.vector.tensor_copy(out=selfA_t[:], in_=selfA_f32[:])

            for _i in range(6):
                zm = mpool.tile([P, bg, D], mdt, tag="msg")
                nc.vector.memset(zm[:], 0.0)
            slab0 = slab_pool.tile([P, nlocp], f32, tag="s0")

            xt_sb = slab_pool.tile([P, nlocp], f32, tag="s0")
            nc.sync.dma_start(out=xt_sb[:], in_=xT[:])

            def allgather_chunk(lp, j):
                _o, _ln = chunk_off[j], chunk_len[j]
                if use_cc:
                    nc.gpsimd.collective_compute(
                        "AllGather", ALU.bypass,
                        ins=[bounce[lp][j][:]],
                        outs=[tbl[lp][N_CORES * _o:N_CORES * (_o + _ln), :]],
                        replica_groups=[list(range(N_CORES))])
                else:
                    nc.sync.dma_start(
                        out=tbl[lp][N_CORES * _o:N_CORES * _o + _ln, :],
                        in_=bounce[lp][j][:])

            def make_producer(lp, from_x=False):
                """per-strip emitter filling bounce[lp] rows (dinv-scaled)
                and firing each chunk's AllGather as soon as its rows are
                staged. lp in {0,1}: h @ W_lp; lp==2: transpose of slab."""
                st = {'stage': None, 'ag': 0}

                def emit(s):
                    lhsT = (xt_sb if from_x else slab0)[:, s * P:(s + 1) * P]
                    ps = gp_pool.tile([P, D], f32, space="PSUM", tag="gps",
                                      name=f"gps_p{lp}_{s}")
                    rhs = selfA_f32[:, 0, :] if lp == 2 else w_t[lp][:]
                    nc.tensor.matmul(ps[:], lhsT, rhs, start=True, stop=True)
                    k = s % 4
                    if k == 0:
                        st['stage'] = stpool.tile([P, 4, D], mdt, tag="gstage",
                                                  name=f"gstage{lp}_{s}")
                    stage = st['stage']
                    nc.scalar.activation(out=stage[:, k, :], in_=ps[:],
                                         func=AF.Copy,
                                         scale=dinvT_t[:, s:s + 1])
                    if k == 3 or s == n_strips - 1:
                        s0 = s - k
                        for (bt, lo, hi) in bounce_rows(lp, s0 * P, (s + 1) * P):
                            dv = bt[lo:hi, :].rearrange("(g p) f -> p g f", p=P)
                            g0 = (chunk_off_of(bt) + lo - s0 * P) // P
                            nc.sync.dma_start(
                                out=dv, in_=stage[:, g0:g0 + (hi - lo) // P, :])
                        while (st['ag'] < len(chunk_len)
                               and chunk_off[st['ag'] + 1] <= (s + 1) * P):
                            allgather_chunk(lp, st['ag'])
                            st['ag'] += 1
                return emit

            seg_counter = [0]

            def message_passing(l, relu_bias, on_strip=None):
                """aggregate tbl[l] messages into slab0 (T layout);
                on_strip(s) is called right after strip s's epilogue."""
                t = tbl[l]
                msg_of_batch = {}
                dst_of_batch = {}
                a_of_batch = {}
                acc_of_strip_d = {}
                dbc_of_strip = {}

                def acc_of_strip(s):
                    return acc_of_strip_d[s][:]

                def ensure_batch(b):
                    if b in msg_of_batch:
                        return
                    mt = mpool.tile([P, bg, D], mdt, tag="msg")
                    msg_of_batch[b] = mt
                    it = spool.tile([P, bg16], i16, tag="idx")
                    nc.sync.dma_start(out=it[:],
                                      in_=idxd[:, b * bg16:(b + 1) * bg16])
                    dt_ = spool.tile([P, bg], f16, tag="dst")
                    nc.sync.dma_start(out=dt_[:],
                                      in_=dstpd[:, b * bg:(b + 1) * bg])
                    dst_of_batch[b] = dt_
                    for (bb, glo, ghi, ck, sid) in segments:
                        if bb != b:
                            continue
                        n_idx = (ghi - glo) * P
                        o8 = N_CORES * chunk_off[ck]
                        l8 = N_CORES * chunk_len[ck]
                        nc.gpsimd.dma_gather(
                            mt[:, glo:ghi, :],
                            t[o8:o8 + l8, :],
                            it[:, glo * 8:ghi * 8],
                            n_idx, n_idx, D, single_packet=False,
                            queue_num=seg_counter[0] % N_QUEUES,
                        )
                        seg_counter[0] += 1

                AB = 8  # groups per batched A-build

                def ensure_abuilds(b):
                    if b in a_of_batch:
                        return
                    dt_ = dst_of_batch[b]
                    tiles = []
                    for w in range(bg // AB):
                        At = apool.tile([P, AB, P], mdt, tag="A")
                        iota_b = bass.AP(iota_t[:].tensor, iota_t[:].offset,
                                         [iota_t[:].ap[0], [0, AB],
                                          iota_t[:].ap[1]])
                        nc.vector.tensor_tensor(
                            out=At[:], in0=iota_b,
                            in1=dt_[:, w * AB:(w + 1) * AB].to_broadcast(
                                [P, AB, P]),
                            op=ALU.is_equal,
                        )
                        tiles.append(At)
                    a_of_batch[b] = tiles

                for (kind, s, b, glo, ghi, stop) in oplist:
                    if kind == 'self':
                        smt = smt_pool.tile([P, 1, D], mdt, tag="smt")
                        for (bt, lo, hi) in bounce_rows(l, s * P, (s + 1) * P):
                            dv = bt[lo:hi, :].rearrange("(g p) f -> p g f", p=P)
                            g0 = (chunk_off_of(bt) + lo - s * P) // P
                            nc.sync.dma_start(
                                out=smt[:, g0:g0 + (hi - lo) // P, :], in_=dv)
                        dbc = dbc_pool.tile([P, P], f32, tag="dbc")
                        nc.sync.dma_start(
                            out=dbc[:], in_=dinvRd[:, s * P:(s + 1) * P])
                        dbc_of_strip[s] = dbc
                        acc_of_strip_d[s] = acc_pool.tile(
                            [P, P], f32, space="PSUM", tag="acc",
                            name=f"acc_s{s}")
                        accv = acc_of_strip(s)
                        pat = 1 if s == n_strips - 1 else 0
                        nc.tensor.matmul(accv, smt[:, 0, :],
                                         selfA_t[:, pat, :],
                                         start=True, stop=stop)
                    else:
                        ensure_batch(b)
                        ensure_abuilds(b)
                        mt = msg_of_batch[b]
                        at = a_of_batch[b]
                        accv = acc_of_strip(s)
                        for g in range(glo, ghi):
                            nc.tensor.matmul(
                                accv, mt[:, g, :], at[g // AB][:, g % AB, :],
                                start=False, stop=(stop and g == ghi - 1),
                            )
                    if stop:
                        accv = acc_of_strip(s)
                        dstv = slab0[:, s * P:(s + 1) * P]
                        nc.vector.tensor_tensor(out=dstv, in0=accv,
                                                in1=dbc_of_strip[s][:],
                                                op=ALU.mult)
                        if relu_bias is not None:
                            nc.scalar.activation(out=dstv, in_=dstv,
                                                 func=AF.Relu,
                                                 bias=relu_bias[:])
                        if on_strip is not None:
                            on_strip(s)

            # fused pipeline: layer l's per-strip epilogues immediately feed
            # layer l+1's produce + chunk AllGathers.
            osb = cpool.tile([P, n_strips], f32, tag="osb")

            def w3_tail(s):
                ps = gp_pool.tile([P, 1], f32, space="PSUM", tag="gps",
                                  name=f"gps_w3_{s}")
                nc.tensor.matmul(
                    ps[:], slab0[:, s * P:(s + 1) * P], w3_t[:],
                    start=True, stop=True)
                nc.vector.tensor_copy(out=osb[:, s:s + 1], in_=ps[:])

            prod0 = make_producer(0, from_x=True)
            for s in range(n_strips):
                prod0(s)
            message_passing(0, b_t[0], on_strip=make_producer(1))
            message_passing(1, b_t[1], on_strip=make_producer(2))
            message_passing(2, None, on_strip=w3_tail)
            nc.scalar.activation(out=osb[:], in_=osb[:],
                                 func=AF.Sigmoid, bias=b3_t[:])
            nc.sync.dma_start(
                out=out[:].rearrange("(s p) one -> p (s one)", p=P),
                in_=osb[:])

    nc.finalize()
    return nc


def make_inputs(meta, x, W1v, b1v, W2v, b2v, W3v, b3v, nloc):
    """Per-core input dicts for run_bass_kernel_spmd."""
    nlocp = meta["nlocp"]
    iota = np.tile(np.arange(P, dtype=np.float16), (P, 1))
    maps = []
    for c in range(N_CORES):
        xl = np.zeros((nlocp, D), np.float32)
        r = x[c * nloc:(c + 1) * nloc]
        xl[:r.shape[0]] = r
        maps.append(dict(
            xT=np.ascontiguousarray(xl.T),
            W1=W1v.astype(np.float32), W2=W2v.astype(np.float32),
            W3=W3v.astype(np.float32).reshape(P, 1),
            b1=b1v.astype(np.float32).reshape(P, 1),
            b2=b2v.astype(np.float32).reshape(P, 1),
            b3=np.full((P, 1), float(b3v.reshape(-1)[0]), np.float32),
            iota=iota,
            idxd=meta["idx_wrapped"][c],
            dstpd=meta["dstp_g"][c].astype(np.float16),
            dinvTd=meta["dinvT"][c],
            dinvRd=np.tile(meta["dinv_row"][c], (P, 1)),
            selfAd=meta["selfA"].reshape(P, 2 * P),
        ))
    return maps


def kernel(x, ei, W1, b1, W2, b2, W3, b3):
    x = np.asarray(x, dtype=np.float32)
    ei_np = np.asarray(ei)
    meta = build(ei_np, N_NODES, NLOC, batch_groups=64)
    nc = build_program(meta)
    in_maps = make_inputs(meta, x,
                          np.asarray(W1), np.asarray(b1),
                          np.asarray(W2), np.asarray(b2),
                          np.asarray(W3), np.asarray(b3), NLOC)
    res = run_bass_kernel_spmd(nc, in_maps, list(range(N_CORES)))
    out = np.concatenate(
        [res.results[c]["out"].reshape(-1)[:NLOC] for c in range(N_CORES)])
    return out.reshape(N_NODES, 1).astype(np.float32)
